# revision 34
# baseline (speedup 1.0000x reference)
"""Child-Sum Tree-LSTM over a complete 4-ary tree on TRN2.

Tree: 21845 nodes, depth 7, branching 4. Leaves (level 7) keep h=c=0, so only
the 5461 internal nodes produce output; rows 5461+ of h/c are zero.

Distribution: the whole tree runs on ONE NeuronCore, dispatched via plain jit
+ bass2jax.fast_dispatch_compile (no shard_map, no collectives). Rationale,
measured in this axon/PJRT environment: an in-NEFF AllGather costs ~250 us of
host-side per-exec overhead and the 8-core shard_map dispatch path another
~100 us — far more than the ~130 us of extra on-device compute that computing
the upper levels' entire subtree locally adds. The no-communication program
is PE-bound at ~220 us (TimelineSim; fp16 matmul FLOP floor is 183 us), and
the 1-core fast-dispatch floor is ~105 us/exec.

On-device layout is transposed ([features, nodes], node columns in natural
index order) so no transposes are needed anywhere. Matmul operands and all
h/c stores are fp16 (full-rate TensorE, ~5e-4 element error); PSUM
accumulation is fp32. x DMAs are per-(k-chunk, col-range) so the Tile hazard
tracker (bounding-box based) gives each level-6 chunk a precise dependency on
exactly its own columns.

Per level (m parents, 4m children, node cols x0..x0+m):
  iou: one PSUM group per 128-row gate tile fuses W_iou@x (child-independent)
       with U_iou@h_sum; a single ScalarE activation applies bias + the
       nonlinearity straight out of PSUM. h_sum/c_sum build incrementally on
       DVE (tensor_reduce over a [*, m, 4] view) as child chunks land.
  forget (no sigmoid, faithful to the source):
       c_new = i*u + (W_f x + b_f) . (sum_k c_ch)  +  sum_k (U_f h_ch) . c_ch
       so W_f@x is computed once per parent (Identity activation folds b_f)
       instead of once per child; the U_f term works on child columns in
       <=512-col PSUM subgroups.
Level 6's children are leaves: iou = Wx only, c = i*u.  Levels 6 and 5 are
processed in two fused halves (2048 l6 nodes feeding 512 l5 parents) so the
l6 h/c stores stay at 2048 columns instead of 4096 (SBUF pressure).
"""

import numpy as np

_H = 512
_NN = 21845
_NI = 5461

_cache = {}


def _build_nc():
    import concourse.bacc as bacc
    import concourse.tile as tile
    import concourse.mybir as mybir

    F32 = mybir.dt.float32
    F16 = mybir.dt.float16
    AF = mybir.ActivationFunctionType
    AXX = mybir.AxisListType.X

    # no collectives and identical work everywhere: no partition-id input
    nc = bacc.Bacc("TRN2", target_bir_lowering=False,
                   enable_partition_id=False)

    xT = nc.declare_dram_parameter("xT", [_H, _NI], F16, isOutput=False)
    # packed weights: cols 0:1536 W_iou.T, 1536:2048 W_f.T,
    # 2048:3584 U_iou.T, 3584:4096 U_f.T
    wT = nc.declare_dram_parameter("wT", [_H, 4096], F16, isOutput=False)
    # packed biases: cols 0:12 b_iou (per 128-row gate tile), 12:16 b_f
    bT = nc.declare_dram_parameter("bT", [128, 16], F32, isOutput=False)
    oh = nc.declare_dram_parameter("oh", [_H, _NI], F16, isOutput=True)
    oc = nc.declare_dram_parameter("oc", [_H, _NI], F16, isOutput=True)

    def dview(p):
        # [F*128, n] DRAM -> [128, F, n] view (feature-chunk-major rows)
        return p.ap().rearrange("(f p) n -> p f n", p=128)

    with tile.TileContext(nc) as tc:
        with (
            tc.tile_pool(name="w", bufs=1) as wpool,
            tc.tile_pool(name="st", bufs=1) as spool,
            tc.tile_pool(name="tmp", bufs=2) as tpool,
            tc.tile_pool(name="psg", bufs=4, space="PSUM") as psg,
            tc.tile_pool(name="psu", bufs=2, space="PSUM") as psu,
        ):
            # --- weights / inputs to SBUF ---
            # critical path first on the sync queue: biases (needed by the
            # first l6 gate ACT), w_iou chunks interleaved with the first l6
            # x columns, then the rest.
            bias = wpool.tile([128, 16], F32)
            nc.sync.dma_start(out=bias[:], in_=bT.ap())
            w_iou = wpool.tile([128, 4, 1536], F16)
            xt = wpool.tile([128, 4, _NI], F16)
            wv = dview(wT)
            xv = dview(xT)
            # x loads are per-(ck, col-range): a single-ck slice is one
            # contiguous range per partition, so the hazard tracker gives
            # each level-6 chunk's matmuls a precise dependency on exactly
            # its own columns (a multi-ck write's strided bounding box
            # overlaps every later read and stalls PE ~16us)
            for ck in range(4):
                nc.sync.dma_start(out=w_iou[:, ck, :], in_=wv[:, ck, 0:1536])
                nc.sync.dma_start(out=xt[:, ck, 1365:1877],
                                  in_=xv[:, ck, 1365:1877])
            for ck in range(4):  # rest of l6 half 0
                nc.sync.dma_start(out=xt[:, ck, 1877:3413],
                                  in_=xv[:, ck, 1877:3413])
            w_f = wpool.tile([128, 4, _H], F16)
            nc.sync.dma_start(out=w_f[:], in_=wv[:, :, 1536:2048])
            for ck in range(4):  # l6 half 1
                nc.scalar.dma_start(out=xt[:, ck, 3413:5461],
                                    in_=xv[:, ck, 3413:5461])
            u_iou = wpool.tile([128, 4, 3 * _H], F16)
            nc.sync.dma_start(out=u_iou[:], in_=wv[:, :, 2048:3584])
            u_f = wpool.tile([128, 4, _H], F16)
            nc.sync.dma_start(out=u_f[:], in_=wv[:, :, 3584:4096])
            for ck in range(4):  # levels 5..0 x columns
                nc.scalar.dma_start(out=xt[:, ck, 0:1365],
                                    in_=xv[:, ck, 0:1365])

            # dummy sigmoid so the ACT function-table load (~1.3us) happens
            # during the DMA phase instead of stalling the first l6 gate
            warm = tpool.tile([128, 1], F16, tag="warm", name="warm", bufs=1)
            nc.vector.memset(warm[:], 0.0)
            nc.scalar.activation(warm[:], warm[:], AF.Sigmoid)

            oh_v, oc_v = dview(oh), dview(oc)

            # --- h/c stores (fp16: h feeds U matmuls) ---
            h6 = spool.tile([128, 4, 2048], F16, tag="h6", name="h6")
            c6 = spool.tile([128, 4, 2048], F16, tag="c6", name="c6")
            h5 = spool.tile([128, 4, 1024], F16, tag="h5", name="h5")
            c5 = spool.tile([128, 4, 1024], F16, tag="c5", name="c5")
            st = {}
            for key, m in (("l4", 256), ("l3", 64), ("l2", 16), ("l1", 4),
                           ("l0", 1)):
                st[key] = (
                    spool.tile([128, 4, m], F16, tag=f"h_{key}",
                               name=f"h_{key}"),
                    spool.tile([128, 4, m], F16, tag=f"c_{key}",
                               name=f"c_{key}"),
                )

            def level6_chunk(g0, h_dst, c_dst):
                """512 level-6 nodes at cols g0..g0+512 (children are leaves:
                iou = Wx only, c = i*u). h_dst/c_dst: [128, 4, 512] slices."""
                g6 = {}
                for f in range(4):
                    for g in range(3):  # 0=i 1=o 2=u
                        mt = 4 * g + f
                        ps = psg.tile([128, 512], F32, tag="ps_g", name="ps")
                        for ck in range(4):
                            nc.tensor.matmul(
                                ps[:], w_iou[:, ck, 128 * mt:128 * (mt + 1)],
                                xt[:, ck, g0:g0 + 512],
                                start=(ck == 0), stop=(ck == 3))
                        gt = tpool.tile([128, 512], F16, tag=f"g6_{g}",
                                        name=f"g6_{g}", bufs=4)
                        nc.scalar.activation(gt[:], ps[:],
                                             AF.Tanh if g == 2 else AF.Sigmoid,
                                             bias=bias[:, mt:mt + 1])
                        g6[(f, g)] = gt
                for f in range(4):
                    nc.vector.tensor_mul(c_dst[:, f, :], g6[(f, 0)][:],
                                         g6[(f, 2)][:])
                    tc6 = tpool.tile([128, 512], F16, tag="tc6", name="tc6")
                    nc.scalar.activation(tc6[:], c_dst[:, f, :], AF.Tanh)
                    nc.vector.tensor_mul(h_dst[:, f, :], g6[(f, 1)][:], tc6[:])
                nc.sync.dma_start(out=oh_v[:, :, g0:g0 + 512], in_=h_dst)
                nc.scalar.dma_start(out=oc_v[:, :, g0:g0 + 512], in_=c_dst)

            def child_sums(child_h, child_c, hsum, csum, o, m):
                """Partial child reductions into hsum/csum cols o..o+m on the
                (children of col p are the 4 cols 4p..4p+4)."""
                hv = child_h.rearrange("p f (m k) -> p f m k", k=4)
                cv = child_c.rearrange("p f (m k) -> p f m k", k=4)
                with nc.allow_low_precision("4-elt child sums"):
                    for ck in range(4):
                        nc.vector.tensor_reduce(hsum[:, ck, o:o + m],
                                                hv[:, ck], AXX,
                                                mybir.AluOpType.add)
                        nc.vector.tensor_reduce(csum[:, ck, o:o + m],
                                                cv[:, ck], AXX,
                                                mybir.AluOpType.add)

            def level(m, x0, child_h, child_c, h_out, c_out,
                      hsum=None, csum=None):
                """One internal level: m parents at cols x0..x0+m, children in
                child_h/child_c ([128, 4, 4m] APs, parent-major columns).
                hsum/csum: precomputed child sums (tiles, cols 0:m), or None
                to compute here."""
                # forget W-side first: child-independent, PE can run it
                # during the previous level's tail
                wxf = tpool.tile([128, 4, 512], F16, tag="wxf", name="wxf")
                for f in range(4):
                    psw = psu.tile([128, 512], F32, tag="ps_wf", name="ps_wf")
                    for ck in range(4):
                        nc.tensor.matmul(psw[:, 0:m],
                                         w_f[:, ck, 128 * f:128 * (f + 1)],
                                         xt[:, ck, x0:x0 + m],
                                         start=(ck == 0), stop=(ck == 3))
                    nc.scalar.activation(wxf[:, f, 0:m], psw[:, 0:m],
                                         AF.Identity,
                                         bias=bias[:, 12 + f:13 + f])

                if hsum is None:
                    hsum = tpool.tile([128, 4, 512], F16, tag="hsum",
                                      name="hsum")
                    csum = tpool.tile([128, 4, 512], F16, tag="csum",
                                      name="csum")
                    child_sums(child_h, child_c, hsum, csum, 0, m)

                # forget U-side: fc = sum_k (U_f @ h_ch) . c_ch over child
                # columns in <=512-col PSUM subgroups
                fc = tpool.tile([128, 4, 512], F16, tag="fc", name="fc")
                for f in range(4):
                    prod = tpool.tile([128, 2048], F16, tag="prod",
                                      name="prod")
                    nsub = max(1, (4 * m) // 512)
                    sub = (4 * m) // nsub
                    for s in range(nsub):
                        pu = psu.tile([128, 512], F32, tag="ps_uf",
                                      name="ps_uf")
                        for ck in range(4):
                            nc.tensor.matmul(
                                pu[:, 0:sub],
                                u_f[:, ck, 128 * f:128 * (f + 1)],
                                child_h[:, ck, sub * s:sub * (s + 1)],
                                start=(ck == 0), stop=(ck == 3))
                        nc.vector.tensor_mul(
                            prod[:, sub * s:sub * (s + 1)], pu[:, 0:sub],
                            child_c[:, f, sub * s:sub * (s + 1)])
                    with nc.allow_low_precision("fc reduce"):
                        nc.vector.tensor_reduce(
                            fc[:, f, 0:m],
                            prod[:, 0:4 * m].rearrange("p (m k) -> p m k",
                                                       k=4),
                            AXX, mybir.AluOpType.add)

                # iou gates: psum = W @ x + U @ hsum, ACT+bias from PSUM
                gt = {}
                for g in range(3):
                    for f in range(4):
                        mt = 4 * g + f
                        ps = psg.tile([128, 512], F32, tag="ps_g", name="ps")
                        for ck in range(4):
                            nc.tensor.matmul(
                                ps[:, 0:m],
                                w_iou[:, ck, 128 * mt:128 * (mt + 1)],
                                xt[:, ck, x0:x0 + m],
                                start=(ck == 0), stop=False)
                        for ck in range(4):
                            nc.tensor.matmul(
                                ps[:, 0:m],
                                u_iou[:, ck, 128 * mt:128 * (mt + 1)],
                                hsum[:, ck, 0:m], start=False, stop=(ck == 3))
                        t = tpool.tile([128, 512], F16, tag=f"g_{g}",
                                       name=f"g_{g}", bufs=4)
                        nc.scalar.activation(t[:, 0:m], ps[:, 0:m],
                                             AF.Tanh if g == 2 else AF.Sigmoid,
                                             bias=bias[:, mt:mt + 1])
                        gt[(f, g)] = t

                for f in range(4):
                    nc.vector.tensor_mul(c_out[:, f, :], gt[(f, 0)][:, 0:m],
                                         gt[(f, 2)][:, 0:m])
                    wxc = tpool.tile([128, 512], F16, tag="wxc", name="wxc")
                    nc.vector.tensor_mul(wxc[:, 0:m], wxf[:, f, 0:m],
                                         csum[:, f, 0:m])
                    nc.vector.tensor_add(c_out[:, f, :], c_out[:, f, :],
                                         wxc[:, 0:m])
                    nc.vector.tensor_add(c_out[:, f, :], c_out[:, f, :],
                                         fc[:, f, 0:m])
                    tct = tpool.tile([128, 512], F16, tag="tct", name="tct")
                    nc.scalar.activation(tct[:, 0:m], c_out[:, f, :], AF.Tanh)
                    nc.vector.tensor_mul(h_out[:, f, :], gt[(f, 1)][:, 0:m],
                                         tct[:, 0:m])

            def dma_out(x0, m, h_t, c_t):
                nc.sync.dma_start(out=oh_v[:, :, x0:x0 + m], in_=h_t)
                nc.scalar.dma_start(out=oc_v[:, :, x0:x0 + m], in_=c_t)

            # ---- levels 6+5, fused in two halves; child sums build
            # incrementally as each l6 chunk lands so the l5 U-matmuls
            # unblock right after the last chunk ----
            hsum4 = tpool.tile([128, 4, 256], F16, tag="hsum4", name="hsum4",
                               bufs=1)
            csum4 = tpool.tile([128, 4, 256], F16, tag="csum4", name="csum4",
                               bufs=1)
            for half in range(2):
                hsum5 = tpool.tile([128, 4, 512], F16, tag="hsum", name="hsum")
                csum5 = tpool.tile([128, 4, 512], F16, tag="csum", name="csum")
                for j in range(4):
                    g0 = 1365 + 2048 * half + 512 * j
                    h6s = h6[:, :, 512 * j:512 * (j + 1)]
                    c6s = c6[:, :, 512 * j:512 * (j + 1)]
                    level6_chunk(g0, h6s, c6s)
                    child_sums(h6s, c6s, hsum5, csum5, 128 * j, 128)
                x0 = 341 + 512 * half
                h5s = h5[:, :, 512 * half:512 * (half + 1)]
                c5s = c5[:, :, 512 * half:512 * (half + 1)]
                level(512, x0, h6[:], c6[:], h5s, c5s, hsum5, csum5)
                dma_out(x0, 512, h5s, c5s)
                child_sums(h5s, c5s, hsum4, csum4, 128 * half, 128)

            # ---- levels 4..0 ----
            level(256, 85, h5[:], c5[:], st["l4"][0][:], st["l4"][1][:],
                  hsum4, csum4)
            dma_out(85, 256, st["l4"][0][:], st["l4"][1][:])
            level(64, 21, st["l4"][0][:], st["l4"][1][:],
                  st["l3"][0][:], st["l3"][1][:])
            dma_out(21, 64, st["l3"][0][:], st["l3"][1][:])
            level(16, 5, st["l3"][0][:], st["l3"][1][:],
                  st["l2"][0][:], st["l2"][1][:])
            dma_out(5, 16, st["l2"][0][:], st["l2"][1][:])
            level(4, 1, st["l2"][0][:], st["l2"][1][:],
                  st["l1"][0][:], st["l1"][1][:])
            dma_out(1, 4, st["l1"][0][:], st["l1"][1][:])
            level(1, 0, st["l1"][0][:], st["l1"][1][:],
                  st["l0"][0][:], st["l0"][1][:])
            dma_out(0, 1, st["l0"][0][:], st["l0"][1][:])

    nc.compile()
    return nc


def _get_nc():
    if "nc" not in _cache:
        _cache["nc"] = _build_nc()
    return _cache["nc"]


def _host_inputs(x, W_iou, b_iou, W_f, b_f, U_iou, U_f):
    """Pack the full (replicated) per-core input map."""
    x = np.asarray(x, np.float32)
    xTk = np.ascontiguousarray(x[0:_NI].T).astype(np.float16)
    wT = np.concatenate([
        np.asarray(W_iou, np.float32).T, np.asarray(W_f, np.float32).T,
        np.asarray(U_iou, np.float32).T, np.asarray(U_f, np.float32).T,
    ], axis=1).astype(np.float16)
    bT = np.concatenate([
        np.asarray(b_iou, np.float32).reshape(12, 128).T,
        np.asarray(b_f, np.float32).reshape(4, 128).T,
    ], axis=1)
    return {"xT": xTk, "wT": np.ascontiguousarray(wT),
            "bT": np.ascontiguousarray(bT)}


def _runner_parts():
    """Compile the module once for single-core fast-path dispatch.

    The program has no collectives and is fully replicated, so it runs on
    one NeuronCore via plain jit (no shard_map) with bass2jax's
    fast_dispatch_compile (C++ no-effect dispatch) — measured ~100us less
    per-exec overhead than the 8-core shard_map path in this axon setup.
    Returns (compiled_fn, in_names, out_names, zero_shapes).
    """
    if "parts" in _cache:
        return _cache["parts"]
    import jax
    import concourse.mybir as mybir
    from concourse import bass2jax

    bass2jax.install_neuronx_cc_hook()
    nc = _get_nc()

    partition_name = (nc.partition_id_tensor.name
                      if nc.partition_id_tensor else None)
    in_names, out_names, out_avals, zero_shapes = [], [], [], []
    for alloc in nc.m.functions[0].allocations:
        if not isinstance(alloc, mybir.MemoryLocationSet):
            continue
        name = alloc.memorylocations[0].name
        if alloc.kind == "ExternalInput":
            if name != partition_name:
                in_names.append(name)
        elif alloc.kind == "ExternalOutput":
            shape = tuple(alloc.tensor_shape)
            dtype = mybir.dt.np(alloc.dtype)
            out_names.append(name)
            out_avals.append(jax.core.ShapedArray(shape, dtype))
            zero_shapes.append((shape, dtype))
    all_names = in_names + out_names
    if partition_name is not None:
        all_names = all_names + [partition_name]

    def _body(*args):
        operands = list(args)
        if partition_name is not None:
            operands.append(bass2jax.partition_id_tensor())
        outs = bass2jax._bass_exec_p.bind(
            *operands, out_avals=tuple(out_avals), in_names=tuple(all_names),
            out_names=tuple(out_names), lowering_input_output_aliases=(),
            sim_require_finite=True, sim_require_nnan=True, nc=nc)
        return tuple(outs)

    in_shapes = {"xT": ((_H, _NI), np.float16),
                 "wT": ((_H, 4096), np.float16),
                 "bT": ((128, 16), np.float32)}
    abstract = ([jax.ShapeDtypeStruct(*in_shapes[k]) for k in in_names]
                + [jax.ShapeDtypeStruct(s, d) for s, d in zero_shapes])
    f = bass2jax.fast_dispatch_compile(
        lambda: jax.jit(_body).lower(*abstract).compile())
    _cache["parts"] = (f, in_names, out_names, zero_shapes)
    return _cache["parts"]


def _get_runner():
    return _runner_parts()[0]


def kernel(x, children, W_iou, b_iou, W_f, b_f, U_iou, U_f):
    f, in_names, out_names, zero_shapes = _runner_parts()
    in_map = _host_inputs(x, W_iou, b_iou, W_f, b_f, U_iou, U_f)
    ins = [in_map[k] for k in in_names]
    zeros = [np.zeros(s, d) for s, d in zero_shapes]
    outs = f(*ins, *zeros)
    res = {name: np.asarray(outs[i]) for i, name in enumerate(out_names)}

    h_full = np.zeros((_NN, _H), dtype=np.float32)
    c_full = np.zeros((_NN, _H), dtype=np.float32)
    h_full[0:_NI] = res["oh"].astype(np.float32).T
    c_full[0:_NI] = res["oc"].astype(np.float32).T
    return h_full, c_full
